# revision 6
# baseline (speedup 1.0000x reference)
"""Trainium2 Bass kernel for nn_AVWDCRNN (2-layer Chebyshev graph-conv GRU).

Strategy (per spec sharding hint): data-parallel over batch B=16 across 8
cores (2 batch elements per core), adjacency/weights replicated; the time
recurrence runs sequentially on-chip.

Algebraic restructuring:
  - Chebyshev supports {I, A, 2A^2-I} folded into powers {I, A, A^2} with
    host-folded weights W'0=W0-W2, W'1=W1, W'2=2*W2.
  - A^T and (A^2)^T stay SBUF-resident in bf16; per-step state matmuls use
    (stationary = state-tile node-partitioned, moving = A^T) so both A and
    A^2 applications share one stationary load and come out
    channel-partitioned, ready for the (small) weight projections.
  - x-dependent gate/candidate contributions are precomputed per layer as
    G/U tensors (one batched pass over all T), so the sequential phase only
    does the state-dependent work.

Self-contained: hardcodes shapes; only imports the system concourse stack.
"""
import sys
import types

for _p in ("/opt/trn_rl_repo",):
    if _p not in sys.path:
        sys.path.insert(0, _p)

import numpy as np
import ml_dtypes

import concourse.bacc as bacc
import concourse.bass as bass
import concourse.mybir as mybir
import concourse.tile as tile
from concourse.bass_utils import run_bass_kernel_spmd

BF16 = mybir.dt.bfloat16
F32 = mybir.dt.float32
AF = mybir.ActivationFunctionType
bf16 = ml_dtypes.bfloat16

# problem constants
B, T, N, D = 16, 24, 2048, 64
L, K = 2, 3
N_CORES = 8
BL = B // N_CORES          # batch per core
BD = BL * D                # 128: (b, d) packed columns
KINDS = {"zh": 0, "rh": 1, "uh": 2, "zx": 3, "rx": 4, "ux": 5}


# ---------------------------------------------------------------- builder --
def build_program(nt=N // 128, T_steps=T):
    """Emit the per-core Bass program. nt = node tiles (N=nt*128)."""
    n = nt * 128
    CW = 512
    chunks = [(c * CW, min((c + 1) * CW, n)) for c in range(-(-n // CW))]

    nc = bacc.Bacc("TRN2", target_bir_lowering=False, debug=True)

    # -- external inputs (per core) --
    x_np = nc.dram_tensor("x_np", [T_steps, 128, nt, 128], BF16, kind="ExternalInput")
    x_cp = nc.dram_tensor("x_cp", [T_steps, 128, n], BF16, kind="ExternalInput")
    x_f32 = nc.dram_tensor("x_f32", [T_steps, 128, nt, 128], F32, kind="ExternalInput")
    S1 = nc.dram_tensor("S1", [128, nt, n], BF16, kind="ExternalInput")
    S2 = nc.dram_tensor("S2", [128, nt, n], BF16, kind="ExternalInput")
    wts = nc.dram_tensor("wts", [128, L * 6 * K, 64], BF16, kind="ExternalInput")
    biases = nc.dram_tensor("biases", [128, L * 3], F32, kind="ExternalInput")
    ident = nc.dram_tensor("ident", [128, 128], BF16, kind="ExternalInput")
    init_cp = nc.dram_tensor("init_cp", [L, 128, n], BF16, kind="ExternalInput")
    init_np = nc.dram_tensor("init_np", [L, 128, nt * 128], BF16, kind="ExternalInput")

    # -- external outputs --
    out_cur = nc.dram_tensor("out_cur", [T_steps, 128, nt, 128], F32, kind="ExternalOutput")
    out_hid = nc.dram_tensor("out_hid", [L, 128, n], F32, kind="ExternalOutput")

    def wslot(l, kind, k):
        return (l * 6 + KINDS[kind]) * K + k

    with tile.TileContext(nc) as tc:
        with (
            tc.tile_pool(name="const", bufs=1) as cpool,
            tc.tile_pool(name="state", bufs=1) as spool,
            tc.tile_pool(name="work", bufs=1) as wpool,
            tc.tile_pool(name="psum", bufs=1, space="PSUM") as ppool,
            tc.tile_pool(name="dram", bufs=1, space="DRAM") as dpool,
        ):
            # ---- persistent constants ----
            s1_sb = cpool.tile([128, nt, n], BF16, name="s1_sb")
            s2_sb = cpool.tile([128, nt, n], BF16, name="s2_sb")
            for jt in range(nt):
                nc.sync.dma_start(s1_sb[:, jt, :], S1[:, jt, :])
                nc.sync.dma_start(s2_sb[:, jt, :], S2[:, jt, :])
            wts_sb = cpool.tile([128, L * 6 * K, 64], BF16, name="wts_sb")
            nc.sync.dma_start(wts_sb[:], wts[:])
            bias_sb = cpool.tile([128, L * 3], F32, name="bias_sb")
            nc.sync.dma_start(bias_sb[:], biases[:])
            id_sb = cpool.tile([128, 128], BF16, name="id_sb")
            nc.sync.dma_start(id_sb[:], ident[:])

            # ---- DRAM scratch ----
            Gz_d = dpool.tile([L, T_steps, 128, n], BF16, name="Gz_d")
            Gr_d = dpool.tile([L, T_steps, 128, n], BF16, name="Gr_d")
            Uu_d = dpool.tile([L, T_steps, 128, n], BF16, name="Uu_d")
            seq_np_d = dpool.tile([T_steps, 128, nt * 128], BF16, name="seq_np_d")
            seq_cp_d = dpool.tile([T_steps, 128, n], BF16, name="seq_cp_d")

            def w_ap(l, kind, k, b):
                sl = wslot(l, kind, k)
                return wts_sb[b * 64:(b + 1) * 64, sl, :]

            def bias_ap(l, j):
                return bias_sb[:, l * 3 + j:l * 3 + j + 1]

            def a_apply(stat_np):
                """u1T = (A @ v)^T, u2T = (A^2 @ v)^T from node-partitioned
                stationary v. Returns two [128, n] f32 psums (CP layout)."""
                p1 = ppool.tile([128, n], F32, tag="ps", name="p1")
                p2 = ppool.tile([128, n], F32, tag="ps", name="p2")
                for jt in range(nt):
                    lhs = stat_np[:, jt, :]
                    st, sp = jt == 0, jt == nt - 1
                    for c0, c1 in chunks:
                        sl = slice(c0, c1)
                        nc.tensor.matmul(p1[:, sl], lhs, s1_sb[:, jt, sl],
                                         start=st, stop=sp, skip_group_check=True)
                    for c0, c1 in chunks:
                        sl = slice(c0, c1)
                        nc.tensor.matmul(p2[:, sl], lhs, s2_sb[:, jt, sl],
                                         start=st, stop=sp, skip_group_check=True)
                return p1, p2

            def project(l, kind_psums, srcs, init_from=None):
                """For each (kind, psum): psum[b*64:(b+1)*64] (+)= sum_k
                W[l,kind,k].T @ srcs[k][b64].  init_from: dict kind->sbuf tile
                added via identity matmul (start=True); else first k starts."""
                for kind, ps in kind_psums:
                    if init_from is not None:
                        g = init_from[kind]
                        for c0, c1 in chunks:
                            sl = slice(c0, c1)
                            nc.tensor.matmul(ps[:, sl], id_sb[:], g[:, sl],
                                             start=True, stop=False,
                                             skip_group_check=True)
                for b in range(2):
                    bsl = slice(b * 64, (b + 1) * 64)
                    for k in range(K):
                        rhs = srcs[k]
                        for kind, ps in kind_psums:
                            st = (init_from is None) and (k == 0)
                            sp = k == K - 1
                            for c0, c1 in chunks:
                                sl = slice(c0, c1)
                                nc.tensor.matmul(ps[bsl, sl], w_ap(l, kind, k, b),
                                                 rhs[bsl, sl], start=st, stop=sp,
                                                 skip_group_check=True)

            def cp_to_np(src_cp, name):
                """Transpose [128(b,ch), n] -> node-partitioned [128, nt, 128]."""
                pt = ppool.tile([128, n], BF16, tag="ps", name=f"pt_{name}")
                for jt in range(nt):
                    sl = slice(jt * 128, (jt + 1) * 128)
                    nc.tensor.transpose(pt[:, sl], src_cp[:, sl], id_sb[:])
                dst = wpool.tile([128, nt, 128], BF16, tag="npbf", bufs=4,
                                 name=f"np_{name}")
                nc.vector.tensor_copy(dst.rearrange("p a b -> p (a b)"), pt[:])
                return dst

            # ================= per-layer phases =================
            for l in range(L):
                # ---- precompute G/U for all t ----
                for t in range(T_steps):
                    if l == 0:
                        xnp = wpool.tile([128, nt, 128], BF16, tag="npbf", bufs=4,
                                         name="xnp")
                        nc.sync.dma_start(xnp[:], x_np[t])
                        xcp = wpool.tile([128, n], BF16, tag="bigbf", bufs=8,
                                         name="xcp")
                        nc.sync.dma_start(xcp[:], x_cp[t])
                    else:
                        xa = wpool.tile([128, nt, 128], BF16, tag="npbf", bufs=4,
                                        name="xa")
                        nc.sync.dma_start(xa[:], x_np[t])
                        sa = wpool.tile([128, nt, 128], BF16, tag="npbf", bufs=4,
                                        name="sa")
                        nc.sync.dma_start(sa[:], seq_np_d[t])
                        xnp = wpool.tile([128, nt, 128], BF16, tag="npbf", bufs=4,
                                         name="xnp")
                        nc.vector.tensor_add(xnp.rearrange("p a b -> p (a b)"),
                                             xa.rearrange("p a b -> p (a b)"),
                                             sa.rearrange("p a b -> p (a b)"))
                        xb = wpool.tile([128, n], BF16, tag="bigbf", bufs=8,
                                        name="xb")
                        nc.sync.dma_start(xb[:], x_cp[t])
                        sb = wpool.tile([128, n], BF16, tag="bigbf", bufs=8,
                                        name="sb")
                        nc.sync.dma_start(sb[:], seq_cp_d[t])
                        xcp = wpool.tile([128, n], BF16, tag="bigbf", bufs=8,
                                         name="xcp")
                        nc.vector.tensor_add(xcp[:], xb[:], sb[:])

                    p1, p2 = a_apply(xnp)
                    y1 = wpool.tile([128, n], BF16, tag="bigbf", bufs=8, name="y1")
                    nc.vector.tensor_copy(y1[:], p1[:])
                    y2 = wpool.tile([128, n], BF16, tag="bigbf", bufs=8, name="y2")
                    nc.scalar.activation(y2[:], p2[:], AF.Copy)

                    pgz = ppool.tile([128, n], F32, tag="ps", name="pgz")
                    pgr = ppool.tile([128, n], F32, tag="ps", name="pgr")
                    project(l, [("zx", pgz), ("rx", pgr)], [xcp, y1, y2])
                    gze = wpool.tile([128, n], BF16, tag="bigbf", bufs=8, name="gze")
                    nc.scalar.activation(gze[:], pgz[:], AF.Identity, bias=bias_ap(l, 0))
                    gre = wpool.tile([128, n], BF16, tag="bigbf", bufs=8, name="gre")
                    nc.scalar.activation(gre[:], pgr[:], AF.Identity, bias=bias_ap(l, 1))
                    nc.sync.dma_start(Gz_d[l, t], gze[:])
                    nc.sync.dma_start(Gr_d[l, t], gre[:])

                    puu = ppool.tile([128, n], F32, tag="ps", name="puu")
                    project(l, [("ux", puu)], [xcp, y1, y2])
                    uue = wpool.tile([128, n], BF16, tag="bigbf", bufs=8, name="uue")
                    nc.scalar.activation(uue[:], puu[:], AF.Identity, bias=bias_ap(l, 2))
                    nc.sync.dma_start(Uu_d[l, t], uue[:])

                # ---- sequential recurrence ----
                h_cp = spool.tile([128, n], BF16, tag="hcp", bufs=2, name="hcp0")
                nc.sync.dma_start(h_cp[:], init_cp[l])
                h_np = wpool.tile([128, nt, 128], BF16, tag="npbf", bufs=4,
                                  name="hnp0")
                nc.sync.dma_start(h_np.rearrange("p a b -> p (a b)"), init_np[l])

                for t in range(T_steps):
                    gz = wpool.tile([128, n], BF16, tag="bigbf", bufs=8, name="gz")
                    nc.sync.dma_start(gz[:], Gz_d[l, t])
                    gr = wpool.tile([128, n], BF16, tag="bigbf", bufs=8, name="gr")
                    nc.sync.dma_start(gr[:], Gr_d[l, t])
                    uu = wpool.tile([128, n], BF16, tag="bigbf", bufs=8, name="uu")
                    nc.sync.dma_start(uu[:], Uu_d[l, t])

                    p1, p2 = a_apply(h_np)
                    u1 = wpool.tile([128, n], BF16, tag="bigbf", bufs=8, name="u1")
                    nc.vector.tensor_copy(u1[:], p1[:])
                    u2 = wpool.tile([128, n], BF16, tag="bigbf", bufs=8, name="u2")
                    nc.scalar.activation(u2[:], p2[:], AF.Copy)

                    pz = ppool.tile([128, n], F32, tag="ps", name="pz")
                    pr = ppool.tile([128, n], F32, tag="ps", name="pr")
                    project(l, [("zh", pz), ("rh", pr)], [h_cp, u1, u2],
                            init_from={"zh": gz, "rh": gr})
                    z = wpool.tile([128, n], BF16, tag="bigbf", bufs=8, name="z")
                    nc.scalar.activation(z[:], pz[:], AF.Sigmoid)
                    r = wpool.tile([128, n], BF16, tag="bigbf", bufs=8, name="r")
                    nc.scalar.activation(r[:], pr[:], AF.Sigmoid)

                    cc = wpool.tile([128, n], BF16, tag="bigbf", bufs=8, name="cc")
                    nc.vector.tensor_mul(cc[:], z[:], h_cp[:])
                    c_np = cp_to_np(cc, "c")

                    p1, p2 = a_apply(c_np)
                    v1 = wpool.tile([128, n], BF16, tag="bigbf", bufs=8, name="v1")
                    nc.vector.tensor_copy(v1[:], p1[:])
                    v2 = wpool.tile([128, n], BF16, tag="bigbf", bufs=8, name="v2")
                    nc.scalar.activation(v2[:], p2[:], AF.Copy)

                    ph = ppool.tile([128, n], F32, tag="ps", name="ph")
                    project(l, [("uh", ph)], [cc, v1, v2], init_from={"uh": uu})
                    hc = wpool.tile([128, n], BF16, tag="bigbf", bufs=8, name="hc")
                    nc.scalar.activation(hc[:], ph[:], AF.Tanh)

                    # h_new = hc + r * (h - hc)
                    d1 = wpool.tile([128, n], BF16, tag="bigbf", bufs=8, name="d1")
                    nc.vector.tensor_sub(d1[:], h_cp[:], hc[:])
                    nc.vector.tensor_mul(d1[:], r[:], d1[:])
                    h_cp_new = spool.tile([128, n], BF16, tag="hcp", bufs=2,
                                          name="hcpn")
                    nc.vector.tensor_add(h_cp_new[:], hc[:], d1[:])
                    h_np_new = cp_to_np(h_cp_new, "h")

                    if l == 0:
                        nc.sync.dma_start(seq_cp_d[t], h_cp_new[:])
                        nc.sync.dma_start(seq_np_d[t],
                                          h_np_new.rearrange("p a b -> p (a b)"))
                    else:
                        for hh in range(2):
                            hsl = slice(hh * (nt // 2), (hh + 1) * (nt // 2))
                            fl = slice(hh * (nt // 2) * 128,
                                       (hh + 1) * (nt // 2) * 128)
                            xf = wpool.tile([128, nt // 2, 128], F32, tag="f32h",
                                            bufs=3, name="xf")
                            nc.sync.dma_start(xf[:], x_f32[t, :, hsl, :])
                            ob = wpool.tile([128, nt // 2, 128], F32, tag="f32h",
                                            bufs=3, name="ob")
                            nc.vector.tensor_add(
                                ob.rearrange("p a b -> p (a b)"),
                                xf.rearrange("p a b -> p (a b)"),
                                h_np_new.rearrange("p a b -> p (a b)")[:, fl])
                            nc.sync.dma_start(out_cur[t, :, hsl, :], ob[:])

                    if t == T_steps - 1:
                        for hh in range(2):
                            osl = slice(hh * (n // 2), (hh + 1) * (n // 2))
                            hl = wpool.tile([128, n // 2], F32, tag="hlast",
                                            bufs=1, name="hl")
                            nc.vector.tensor_copy(hl[:], h_cp_new[:, osl])
                            nc.sync.dma_start(out_hid[l][:, osl], hl[:])

                    h_cp = h_cp_new
                    h_np = h_np_new

    nc.compile()
    return nc


# ------------------------------------------------------------- host side --
def fold_cheb(W):
    """[K, cin, cout] Chebyshev -> power-basis weights."""
    return np.stack([W[0] - W[2], W[1], 2.0 * W[2]], axis=0)


def prep_shared(adj, Wg, bg, Wu, bu, nt=N // 128):
    n = nt * 128
    A = np.asarray(adj, np.float32)
    A2 = (A @ A).astype(np.float32)
    S1h = np.ascontiguousarray(
        A.T.reshape(nt, 128, n).transpose(1, 0, 2)).astype(bf16)
    S2h = np.ascontiguousarray(
        A2.T.reshape(nt, 128, n).transpose(1, 0, 2)).astype(bf16)

    Wg = np.asarray(Wg, np.float32)
    Wu = np.asarray(Wu, np.float32)
    wts = np.zeros((128, L * 6 * K, 64), np.float32)
    for l in range(L):
        Wgf, Wuf = fold_cheb(Wg[l]), fold_cheb(Wu[l])
        for k in range(K):
            sl = {
                "zh": Wgf[k][D:, :D], "rh": Wgf[k][D:, D:], "uh": Wuf[k][D:, :],
                "zx": Wgf[k][:D, :D], "rx": Wgf[k][:D, D:], "ux": Wuf[k][:D, :],
            }
            for kind, w in sl.items():
                j = (l * 6 + KINDS[kind]) * K + k
                wts[0:64, j, :] = w
                wts[64:128, j, :] = w
    wtsh = wts.astype(bf16)

    bg = np.asarray(bg, np.float32)
    bu = np.asarray(bu, np.float32)
    biases = np.zeros((128, L * 3), np.float32)
    for l in range(L):
        biases[:, l * 3 + 0] = np.tile(bg[l][:D], 2)
        biases[:, l * 3 + 1] = np.tile(bg[l][D:], 2)
        biases[:, l * 3 + 2] = np.tile(bu[l], 2)

    identh = np.eye(128, dtype=np.float32).astype(bf16)
    return dict(S1=S1h, S2=S2h, wts=wtsh, biases=biases, ident=identh)


def prep_core(xc, init_c, shared, nt=N // 128, T_steps=T):
    """xc: [BL, T_steps, n, D] f32; init_c: [L, BL, n, D] f32."""
    n = nt * 128
    xc = np.asarray(xc, np.float32)
    # NP: [t, p, jt, (b, d)]
    xnp = np.ascontiguousarray(
        xc.reshape(BL, T_steps, nt, 128, D).transpose(1, 3, 2, 0, 4)
        .reshape(T_steps, 128, nt, 128))
    # CP: [t, (b, d), node]
    xcp = np.ascontiguousarray(
        xc.transpose(1, 0, 3, 2).reshape(T_steps, 128, n))
    ic = np.asarray(init_c, np.float32)
    init_cp = np.ascontiguousarray(
        ic.transpose(0, 1, 3, 2).reshape(L, 128, n)).astype(bf16)
    init_np = np.ascontiguousarray(
        ic.reshape(L, BL, nt, 128, D).transpose(0, 3, 2, 1, 4)
        .reshape(L, 128, nt * 128)).astype(bf16)
    m = dict(shared)
    m.update(
        x_np=xnp.astype(bf16),
        x_cp=xcp.astype(bf16),
        x_f32=xnp,
        init_cp=init_cp,
        init_np=init_np,
    )
    return m


def decode_outputs(results, nt=N // 128, T_steps=T):
    """results: list per core of {out_cur, out_hid} -> (current, hiddens)."""
    n = nt * 128
    curs, hids = [], []
    for res in results:
        oc = np.asarray(res["out_cur"], np.float32)  # [t, p, jt, (b,d)]
        cur = (oc.reshape(T_steps, 128, nt, BL, D)
               .transpose(3, 0, 2, 1, 4).reshape(BL, T_steps, n, D))
        curs.append(cur)
        oh = np.asarray(res["out_hid"], np.float32)  # [L, (b,ch), node]
        hid = oh.reshape(L, BL, D, n).transpose(0, 1, 3, 2)  # [L, BL, n, D]
        hids.append(hid)
    current = np.concatenate(curs, axis=0)           # [B, T, N, D]
    hiddens = np.concatenate(hids, axis=1)           # [L, B, N, D]
    return current, hiddens


_NC_CACHE = {}


def get_program(nt=N // 128, T_steps=T):
    key = (nt, T_steps)
    if key not in _NC_CACHE:
        _NC_CACHE[key] = build_program(nt, T_steps)
    return _NC_CACHE[key]


def make_in_maps(x, init_state, adj_matrix, Wg, bg, Wu, bu,
                 nt=N // 128, T_steps=T):
    shared = prep_shared(adj_matrix, Wg, bg, Wu, bu, nt)
    x = np.asarray(x, np.float32)
    ist = np.asarray(init_state, np.float32)
    in_maps = []
    for c in range(N_CORES):
        xc = x[c * BL:(c + 1) * BL]
        ic = ist[:, c * BL:(c + 1) * BL]
        in_maps.append(prep_core(xc, ic, shared, nt, T_steps))
    return in_maps


def kernel(x, init_state, adj_matrix, Wg, bg, Wu, bu):
    nc = get_program()
    in_maps = make_in_maps(x, init_state, adj_matrix, Wg, bg, Wu, bu)
    res = run_bass_kernel_spmd(nc, in_maps, core_ids=list(range(N_CORES)))
    return decode_outputs(res.results)


# optional: traced run for profiling (used by test.py)
def run_traced(x, init_state, adj_matrix, Wg, bg, Wu, bu, tmpdir=None):
    import antenv
    from trn_agent_boot.trn_boot import _ntff_profile_via_ctypes
    hook = _ntff_profile_via_ctypes('/opt/axon/libaxon_pjrt.so')
    mod = types.ModuleType('antenv.axon_hooks')
    mod.get_axon_ntff_profile_hook = lambda: hook
    sys.modules['antenv.axon_hooks'] = mod
    antenv.axon_hooks = mod
    nc = get_program()
    in_maps = make_in_maps(x, init_state, adj_matrix, Wg, bg, Wu, bu)
    res = run_bass_kernel_spmd(nc, in_maps, core_ids=list(range(N_CORES)),
                               trace=True, tmpdir=tmpdir)
    return decode_outputs(res.results), res


# revision 7
# speedup vs baseline: 1.7809x; 1.7809x over previous
"""Trainium2 Bass kernel for nn_AVWDCRNN (2-layer Chebyshev graph-conv GRU).

Strategy (per spec sharding hint): data-parallel over batch B=16 across 8
cores (2 batch elements per core), adjacency/weights replicated; the time
recurrence runs sequentially on-chip.

Algebraic restructuring:
  - Chebyshev supports {I, A, 2A^2-I} folded into powers {I, A, A^2} with
    host-folded weights W'0=W0-W2, W'1=W1, W'2=2*W2.
  - A^T and (A^2)^T stay SBUF-resident in bf16; per-step state matmuls use
    (stationary = state-tile node-partitioned, moving = A^T) so both A and
    A^2 applications share stationary loads and come out channel-partitioned
    ready for the (small) weight projections.
  - x-dependent gate/candidate contributions are precomputed per layer as
    G/U tensors; the precompute for step t+PRE is emitted interleaved with
    sequential step t so its matmuls fill PE gaps (keeps HAM warm).
  - Node dim processed in 1024-column halves so PSUM turns over quickly and
    evictions/projections pipeline against the next half's matmuls.
  - Redundant LDWEIGHTS (same stationary reloaded back-to-back) are removed
    by a post-compile pass.

Self-contained: hardcodes shapes; only imports the system concourse stack.
"""
import sys
import types

for _p in ("/opt/trn_rl_repo",):
    if _p not in sys.path:
        sys.path.insert(0, _p)

import numpy as np
import ml_dtypes

import concourse.bacc as bacc
import concourse.bass as bass
import concourse.mybir as mybir
import concourse.tile as tile
from concourse.bass_utils import run_bass_kernel_spmd

BF16 = mybir.dt.bfloat16
F32 = mybir.dt.float32
AF = mybir.ActivationFunctionType
bf16 = ml_dtypes.bfloat16

# problem constants
B, T, N, D = 16, 24, 2048, 64
L, K = 2, 3
N_CORES = 8
BL = B // N_CORES          # batch per core
BD = BL * D                # 128: (b, d) packed columns
PRE = 3                    # precompute lookahead (steps)
KINDS = {"zh": 0, "rh": 1, "uh": 2, "zx": 3, "rx": 4, "ux": 5}


def dedup_ldweights(nc):
    """Remove back-to-back InstLdweights with identical physical APs (the
    PE keeps the stationary operand loaded across matmuls)."""
    removed = 0
    for f in nc.m.functions:
        for blk in f.blocks:
            out = []
            last = None
            for i in blk.instructions:
                tn = type(i).__name__
                if tn == "InstLdweights":
                    key = repr(i.ins[0])
                    if key == last and not (i.has_wait() or i.has_update()):
                        removed += 1
                        continue
                    last = key
                out.append(i)
            blk.instructions = out
    return removed


# ---------------------------------------------------------------- builder --
def build_program(nt=N // 128, T_steps=T):
    """Emit the per-core Bass program. nt = node tiles (N=nt*128)."""
    n = nt * 128
    HW = min(1024, n)
    halves = [(h, min(h + HW, n)) for h in range(0, n, HW)]

    def ch512(c0, c1):
        return [(a, min(a + 512, c1)) for a in range(c0, c1, 512)]

    nc = bacc.Bacc("TRN2", target_bir_lowering=False, debug=True)

    # -- external inputs (per core) --
    x_np = nc.dram_tensor("x_np", [T_steps, 128, nt, 128], BF16, kind="ExternalInput")
    x_cp = nc.dram_tensor("x_cp", [T_steps, 128, n], BF16, kind="ExternalInput")
    x_f32 = nc.dram_tensor("x_f32", [T_steps, 128, nt, 128], F32, kind="ExternalInput")
    S1 = nc.dram_tensor("S1", [128, nt, n], BF16, kind="ExternalInput")
    S2 = nc.dram_tensor("S2", [128, nt, n], BF16, kind="ExternalInput")
    wts = nc.dram_tensor("wts", [128, L * 6 * K, 64], BF16, kind="ExternalInput")
    biases = nc.dram_tensor("biases", [128, L * 3], F32, kind="ExternalInput")
    ident = nc.dram_tensor("ident", [128, 128], BF16, kind="ExternalInput")
    init_cp = nc.dram_tensor("init_cp", [L, 128, n], BF16, kind="ExternalInput")
    init_np = nc.dram_tensor("init_np", [L, 128, nt * 128], BF16, kind="ExternalInput")

    # -- external outputs --
    out_cur = nc.dram_tensor("out_cur", [T_steps, 128, nt, 128], F32, kind="ExternalOutput")
    out_hid = nc.dram_tensor("out_hid", [L, 128, n], F32, kind="ExternalOutput")

    def wslot(l, kind, k):
        return (l * 6 + KINDS[kind]) * K + k

    with tile.TileContext(nc) as tc:
        with (
            tc.tile_pool(name="const", bufs=1) as cpool,
            tc.tile_pool(name="state", bufs=1) as spool,
            tc.tile_pool(name="work", bufs=1) as wpool,
            tc.tile_pool(name="psum", bufs=1, space="PSUM") as ppool,
            tc.tile_pool(name="dram", bufs=1, space="DRAM") as dpool,
        ):
            # ---- persistent constants ----
            s1_sb = cpool.tile([128, nt, n], BF16, name="s1_sb")
            s2_sb = cpool.tile([128, nt, n], BF16, name="s2_sb")
            wts_sb = cpool.tile([128, L * 6 * K, 64], BF16, name="wts_sb")
            nc.sync.dma_start(wts_sb[:], wts[:])
            bias_sb = cpool.tile([128, L * 3], F32, name="bias_sb")
            nc.sync.dma_start(bias_sb[:], biases[:])
            id_sb = cpool.tile([128, 128], BF16, name="id_sb")
            nc.sync.dma_start(id_sb[:], ident[:])
            for jt in range(nt):
                nc.sync.dma_start(s1_sb[:, jt, :], S1[:, jt, :])
                nc.sync.dma_start(s2_sb[:, jt, :], S2[:, jt, :])

            # ---- DRAM scratch ----
            Gz_d = dpool.tile([L, T_steps, 128, n], BF16, name="Gz_d")
            Gr_d = dpool.tile([L, T_steps, 128, n], BF16, name="Gr_d")
            Uu_d = dpool.tile([L, T_steps, 128, n], BF16, name="Uu_d")
            seq_np_d = dpool.tile([T_steps, 128, nt * 128], BF16, name="seq_np_d")
            seq_cp_d = dpool.tile([T_steps, 128, n], BF16, name="seq_cp_d")

            def wt(name):
                return wpool.tile([128, n], BF16, tag="bigbf", bufs=8, name=name)

            def npt(name):
                return wpool.tile([128, nt, 128], BF16, tag="npbf", bufs=4, name=name)

            def hft(name, w):
                return wpool.tile([128, w], BF16, tag="hfbf", bufs=4, name=name)

            def w_ap(l, kind, k, b):
                sl = wslot(l, kind, k)
                return wts_sb[b * 64:(b + 1) * 64, sl, :]

            def bias_ap(l, j):
                return bias_sb[:, l * 3 + j:l * 3 + j + 1]

            def seq_apply(stat_np, u1, u2):
                """u1 <- (A v)^T, u2 <- (A^2 v)^T (bf16 SBUF, CP layout)."""
                for (c0, c1) in halves:
                    p1 = ppool.tile([128, c1 - c0], F32, tag="psS", bufs=3,
                                    name="p1")
                    p2 = ppool.tile([128, c1 - c0], F32, tag="psS", bufs=3,
                                    name="p2")
                    for jt in range(nt):
                        st, sp = jt == 0, jt == nt - 1
                        lhs = stat_np[:, jt, :]
                        for (a0, a1) in ch512(c0, c1):
                            nc.tensor.matmul(p1[:, a0 - c0:a1 - c0], lhs,
                                             s1_sb[:, jt, a0:a1], start=st,
                                             stop=sp, skip_group_check=True)
                        for (a0, a1) in ch512(c0, c1):
                            nc.tensor.matmul(p2[:, a0 - c0:a1 - c0], lhs,
                                             s2_sb[:, jt, a0:a1], start=st,
                                             stop=sp, skip_group_check=True)
                    nc.vector.tensor_copy(u1[:, c0:c1], p1[:])
                    nc.scalar.activation(u2[:, c0:c1], p2[:], AF.Copy)

            def proj(l, kind, src3, gsb, dst, fn):
                """dst = fn(gsb + sum_k W[l,kind,k].T @ src3[k]) per batch."""
                for (c0, c1) in halves:
                    p = ppool.tile([128, c1 - c0], F32, tag="psS", bufs=3,
                                   name=f"p_{kind}")
                    for (a0, a1) in ch512(c0, c1):
                        nc.tensor.matmul(p[:, a0 - c0:a1 - c0], id_sb[:],
                                         gsb[:, a0:a1], start=True, stop=False,
                                         skip_group_check=True)
                    for k in range(K):
                        for (a0, a1) in ch512(c0, c1):
                            for b in range(2):
                                bsl = slice(b * 64, (b + 1) * 64)
                                nc.tensor.matmul(
                                    p[bsl, a0 - c0:a1 - c0], w_ap(l, kind, k, b),
                                    src3[k][bsl, a0:a1], start=False,
                                    stop=(k == K - 1), skip_group_check=True)
                    nc.scalar.activation(dst[:, c0:c1], p[:], fn)

            def pre_apply(stat_np, y1, y2):
                """Precompute-stream A applies through the single psP slot."""
                for (Ssb, dst, eng) in ((s1_sb, y1, 0), (s2_sb, y2, 1)):
                    for (c0, c1) in halves:
                        p = ppool.tile([128, c1 - c0], F32, tag="psP", bufs=1,
                                       name="pa")
                        for jt in range(nt):
                            st, sp = jt == 0, jt == nt - 1
                            for (a0, a1) in ch512(c0, c1):
                                nc.tensor.matmul(p[:, a0 - c0:a1 - c0],
                                                 stat_np[:, jt, :],
                                                 Ssb[:, jt, a0:a1], start=st,
                                                 stop=sp, skip_group_check=True)
                        if eng == 0:
                            nc.vector.tensor_copy(dst[:, c0:c1], p[:])
                        else:
                            nc.scalar.activation(dst[:, c0:c1], p[:], AF.Copy)

            def pre_proj(l, kind, src3, bias_j, dst_dram, t):
                for (c0, c1) in halves:
                    p = ppool.tile([128, c1 - c0], F32, tag="psP", bufs=1,
                                   name=f"pp_{kind}")
                    for k in range(K):
                        for (a0, a1) in ch512(c0, c1):
                            for b in range(2):
                                bsl = slice(b * 64, (b + 1) * 64)
                                nc.tensor.matmul(
                                    p[bsl, a0 - c0:a1 - c0], w_ap(l, kind, k, b),
                                    src3[k][bsl, a0:a1], start=(k == 0),
                                    stop=(k == K - 1), skip_group_check=True)
                    ge = hft(f"ge_{kind}", c1 - c0)
                    nc.scalar.activation(ge[:], p[:], AF.Identity,
                                         bias=bias_ap(l, bias_j))
                    nc.sync.dma_start(dst_dram[l, t][:, c0:c1], ge[:])

            def cp_to_np_half(src_cp, dst_np, c0, c1):
                pt = ppool.tile([128, c1 - c0], BF16, tag="psS", bufs=3,
                                name="pt")
                for j, jt in enumerate(range(c0 // 128, c1 // 128)):
                    sl = slice(jt * 128, (jt + 1) * 128)
                    nc.tensor.transpose(pt[:, j * 128:(j + 1) * 128],
                                        src_cp[:, sl], id_sb[:])
                nc.vector.tensor_copy(
                    dst_np.rearrange("p a b -> p (a b)")[:, c0:c1], pt[:])

            def emit_pre(l, t):
                xnp = npt("xnp")
                nc.sync.dma_start(xnp[:], x_np[t])
                xcp = wt("xcp")
                nc.sync.dma_start(xcp[:], x_cp[t])
                if l == 1:
                    sa = npt("sa")
                    nc.sync.dma_start(sa[:], seq_np_d[t])
                    nc.vector.tensor_add(xnp.rearrange("p a b -> p (a b)"),
                                         xnp.rearrange("p a b -> p (a b)"),
                                         sa.rearrange("p a b -> p (a b)"))
                    sb_ = wt("sb_")
                    nc.sync.dma_start(sb_[:], seq_cp_d[t])
                    nc.vector.tensor_add(xcp[:], xcp[:], sb_[:])
                y1 = wt("y1")
                y2 = wt("y2")
                pre_apply(xnp, y1, y2)
                src3 = [xcp, y1, y2]
                pre_proj(l, "zx", src3, 0, Gz_d, t)
                pre_proj(l, "rx", src3, 1, Gr_d, t)
                pre_proj(l, "ux", src3, 2, Uu_d, t)

            def emit_step(l, t, h_cp, h_np, last):
                gz = wt("gz")
                nc.sync.dma_start(gz[:], Gz_d[l, t])
                gr = wt("gr")
                nc.sync.dma_start(gr[:], Gr_d[l, t])
                uu = wt("uu")
                nc.sync.dma_start(uu[:], Uu_d[l, t])

                u1 = wt("u1")
                u2 = wt("u2")
                seq_apply(h_np, u1, u2)
                z = wt("z")
                proj(l, "zh", [h_cp, u1, u2], gz, z, AF.Sigmoid)
                r = wt("r")
                proj(l, "rh", [h_cp, u1, u2], gr, r, AF.Sigmoid)

                cc = wt("cc")
                for (c0, c1) in halves:
                    nc.vector.tensor_mul(cc[:, c0:c1], z[:, c0:c1],
                                         h_cp[:, c0:c1])
                c_np = npt("c_np")
                for (c0, c1) in halves:
                    cp_to_np_half(cc, c_np, c0, c1)

                v1 = wt("v1")
                v2 = wt("v2")
                seq_apply(c_np, v1, v2)
                hc = wt("hc")
                proj(l, "uh", [cc, v1, v2], uu, hc, AF.Tanh)

                # h_new = hc + r * (h - hc)
                h_cp_new = spool.tile([128, n], BF16, tag="hcp", bufs=2,
                                      name="hcpn")
                for (c0, c1) in halves:
                    d1 = hft("d1", c1 - c0)
                    nc.vector.tensor_sub(d1[:], h_cp[:, c0:c1], hc[:, c0:c1])
                    nc.vector.tensor_mul(d1[:], r[:, c0:c1], d1[:])
                    nc.vector.tensor_add(h_cp_new[:, c0:c1], hc[:, c0:c1],
                                         d1[:])
                h_np_new = npt("h_npn")
                for (c0, c1) in halves:
                    cp_to_np_half(h_cp_new, h_np_new, c0, c1)

                if l == 0:
                    nc.sync.dma_start(seq_cp_d[t], h_cp_new[:])
                    nc.sync.dma_start(seq_np_d[t],
                                      h_np_new.rearrange("p a b -> p (a b)"))
                else:
                    nh = max(1, nt // 2)
                    for hh in range(nt // nh):
                        hsl = slice(hh * nh, (hh + 1) * nh)
                        fl = slice(hh * nh * 128, (hh + 1) * nh * 128)
                        xf = wpool.tile([128, nh, 128], F32, tag="f32h",
                                        bufs=2, name="xf")
                        nc.sync.dma_start(xf[:], x_f32[t, :, hsl, :])
                        ob = wpool.tile([128, nh, 128], F32, tag="f32h",
                                        bufs=2, name="ob")
                        nc.vector.tensor_add(
                            ob.rearrange("p a b -> p (a b)"),
                            xf.rearrange("p a b -> p (a b)"),
                            h_np_new.rearrange("p a b -> p (a b)")[:, fl])
                        nc.sync.dma_start(out_cur[t, :, hsl, :], ob[:])

                if last:
                    for (c0, c1) in halves:
                        hl = wpool.tile([128, c1 - c0], F32, tag="f32h",
                                        bufs=2, name="hl")
                        nc.vector.tensor_copy(hl[:], h_cp_new[:, c0:c1])
                        nc.sync.dma_start(out_hid[l][:, c0:c1], hl[:])

                return h_cp_new, h_np_new

            # ================= per-layer phases =================
            for l in range(L):
                npre = min(PRE, T_steps)
                for t in range(npre):
                    emit_pre(l, t)
                h_cp = spool.tile([128, n], BF16, tag="hcp", bufs=2,
                                  name="hcp0")
                nc.sync.dma_start(h_cp[:], init_cp[l])
                h_np = npt("hnp0")
                nc.sync.dma_start(h_np.rearrange("p a b -> p (a b)"),
                                  init_np[l])
                for t in range(T_steps):
                    h_cp, h_np = emit_step(l, t, h_cp, h_np,
                                           t == T_steps - 1)
                    if t + npre < T_steps:
                        emit_pre(l, t + npre)

    nc.compile()
    ndup = dedup_ldweights(nc)
    print(f"[kernel] dedup removed {ndup} LDWEIGHTS", file=sys.stderr)
    return nc


# ------------------------------------------------------------- host side --
def fold_cheb(W):
    """[K, cin, cout] Chebyshev -> power-basis weights."""
    return np.stack([W[0] - W[2], W[1], 2.0 * W[2]], axis=0)


def prep_shared(adj, Wg, bg, Wu, bu, nt=N // 128):
    n = nt * 128
    A = np.asarray(adj, np.float32)
    A2 = (A @ A).astype(np.float32)
    S1h = np.ascontiguousarray(
        A.T.reshape(nt, 128, n).transpose(1, 0, 2)).astype(bf16)
    S2h = np.ascontiguousarray(
        A2.T.reshape(nt, 128, n).transpose(1, 0, 2)).astype(bf16)

    Wg = np.asarray(Wg, np.float32)
    Wu = np.asarray(Wu, np.float32)
    wts = np.zeros((128, L * 6 * K, 64), np.float32)
    for l in range(L):
        Wgf, Wuf = fold_cheb(Wg[l]), fold_cheb(Wu[l])
        for k in range(K):
            sl = {
                "zh": Wgf[k][D:, :D], "rh": Wgf[k][D:, D:], "uh": Wuf[k][D:, :],
                "zx": Wgf[k][:D, :D], "rx": Wgf[k][:D, D:], "ux": Wuf[k][:D, :],
            }
            for kind, w in sl.items():
                j = (l * 6 + KINDS[kind]) * K + k
                wts[0:64, j, :] = w
                wts[64:128, j, :] = w
    wtsh = wts.astype(bf16)

    bg = np.asarray(bg, np.float32)
    bu = np.asarray(bu, np.float32)
    biases = np.zeros((128, L * 3), np.float32)
    for l in range(L):
        biases[:, l * 3 + 0] = np.tile(bg[l][:D], 2)
        biases[:, l * 3 + 1] = np.tile(bg[l][D:], 2)
        biases[:, l * 3 + 2] = np.tile(bu[l], 2)

    identh = np.eye(128, dtype=np.float32).astype(bf16)
    return dict(S1=S1h, S2=S2h, wts=wtsh, biases=biases, ident=identh)


def prep_core(xc, init_c, shared, nt=N // 128, T_steps=T):
    """xc: [BL, T_steps, n, D] f32; init_c: [L, BL, n, D] f32."""
    n = nt * 128
    xc = np.asarray(xc, np.float32)
    # NP: [t, p, jt, (b, d)]
    xnp = np.ascontiguousarray(
        xc.reshape(BL, T_steps, nt, 128, D).transpose(1, 3, 2, 0, 4)
        .reshape(T_steps, 128, nt, 128))
    # CP: [t, (b, d), node]
    xcp = np.ascontiguousarray(
        xc.transpose(1, 0, 3, 2).reshape(T_steps, 128, n))
    ic = np.asarray(init_c, np.float32)
    init_cp_h = np.ascontiguousarray(
        ic.transpose(0, 1, 3, 2).reshape(L, 128, n)).astype(bf16)
    init_np_h = np.ascontiguousarray(
        ic.reshape(L, BL, nt, 128, D).transpose(0, 3, 2, 1, 4)
        .reshape(L, 128, nt * 128)).astype(bf16)
    m = dict(shared)
    m.update(
        x_np=xnp.astype(bf16),
        x_cp=xcp.astype(bf16),
        x_f32=xnp,
        init_cp=init_cp_h,
        init_np=init_np_h,
    )
    return m


def decode_outputs(results, nt=N // 128, T_steps=T):
    """results: list per core of {out_cur, out_hid} -> (current, hiddens)."""
    n = nt * 128
    curs, hids = [], []
    for res in results:
        oc = np.asarray(res["out_cur"], np.float32)  # [t, p, jt, (b,d)]
        cur = (oc.reshape(T_steps, 128, nt, BL, D)
               .transpose(3, 0, 2, 1, 4).reshape(BL, T_steps, n, D))
        curs.append(cur)
        oh = np.asarray(res["out_hid"], np.float32)  # [L, (b,ch), node]
        hid = oh.reshape(L, BL, D, n).transpose(0, 1, 3, 2)  # [L, BL, n, D]
        hids.append(hid)
    current = np.concatenate(curs, axis=0)           # [B, T, N, D]
    hiddens = np.concatenate(hids, axis=1)           # [L, B, N, D]
    return current, hiddens


_NC_CACHE = {}


def get_program(nt=N // 128, T_steps=T):
    key = (nt, T_steps)
    if key not in _NC_CACHE:
        _NC_CACHE[key] = build_program(nt, T_steps)
    return _NC_CACHE[key]


def make_in_maps(x, init_state, adj_matrix, Wg, bg, Wu, bu,
                 nt=N // 128, T_steps=T):
    shared = prep_shared(adj_matrix, Wg, bg, Wu, bu, nt)
    x = np.asarray(x, np.float32)
    ist = np.asarray(init_state, np.float32)
    in_maps = []
    for c in range(N_CORES):
        xc = x[c * BL:(c + 1) * BL]
        ic = ist[:, c * BL:(c + 1) * BL]
        in_maps.append(prep_core(xc, ic, shared, nt, T_steps))
    return in_maps


def kernel(x, init_state, adj_matrix, Wg, bg, Wu, bu):
    nc = get_program()
    in_maps = make_in_maps(x, init_state, adj_matrix, Wg, bg, Wu, bu)
    res = run_bass_kernel_spmd(nc, in_maps, core_ids=list(range(N_CORES)))
    return decode_outputs(res.results)


# optional: traced run for profiling (used by test.py)
def run_traced(x, init_state, adj_matrix, Wg, bg, Wu, bu, tmpdir=None):
    import antenv
    from trn_agent_boot.trn_boot import _ntff_profile_via_ctypes
    hook = _ntff_profile_via_ctypes('/opt/axon/libaxon_pjrt.so')
    mod = types.ModuleType('antenv.axon_hooks')
    mod.get_axon_ntff_profile_hook = lambda: hook
    sys.modules['antenv.axon_hooks'] = mod
    antenv.axon_hooks = mod
    nc = get_program()
    in_maps = make_in_maps(x, init_state, adj_matrix, Wg, bg, Wu, bu)
    res = run_bass_kernel_spmd(nc, in_maps, core_ids=list(range(N_CORES)),
                               trace=True, tmpdir=tmpdir)
    return decode_outputs(res.results), res


# revision 9
# speedup vs baseline: 1.8346x; 1.0302x over previous
"""Trainium2 Bass kernel for nn_AVWDCRNN (2-layer Chebyshev graph-conv GRU).

Strategy (per spec sharding hint): data-parallel over batch B=16 across 8
cores (2 batch elements per core), adjacency/weights replicated; the time
recurrence runs sequentially on-chip.

Algebraic restructuring:
  - Chebyshev supports {I, A, 2A^2-I} folded into powers {I, A, A^2} with
    host-folded weights W'0=W0-W2, W'1=W1, W'2=2*W2.
  - A^T and (A^2)^T stay SBUF-resident in bf16; per-step state matmuls use
    (stationary = state-tile node-partitioned, moving = A^T) so both A and
    A^2 applications share stationary loads and come out channel-partitioned
    ready for the (small) weight projections.
  - x-dependent gate/candidate contributions are precomputed per layer as
    G/U tensors; the precompute for step t+PRE is emitted interleaved with
    sequential step t so its matmuls fill PE gaps (keeps HAM warm).
  - Node dim processed in 1024-column halves so PSUM turns over quickly and
    evictions/projections pipeline against the next half's matmuls.
  - Redundant LDWEIGHTS (same stationary reloaded back-to-back) are removed
    by a post-compile pass.

Self-contained: hardcodes shapes; only imports the system concourse stack.
"""
import sys
import types

for _p in ("/opt/trn_rl_repo",):
    if _p not in sys.path:
        sys.path.insert(0, _p)

import numpy as np
import ml_dtypes

import concourse.bacc as bacc
import concourse.bass as bass
import concourse.mybir as mybir
import concourse.tile as tile
from concourse.bass_utils import run_bass_kernel_spmd

BF16 = mybir.dt.bfloat16
F32 = mybir.dt.float32
AF = mybir.ActivationFunctionType
bf16 = ml_dtypes.bfloat16

# problem constants
B, T, N, D = 16, 24, 2048, 64
L, K = 2, 3
N_CORES = 8
BL = B // N_CORES          # batch per core
BD = BL * D                # 128: (b, d) packed columns
PRE = 3                    # precompute lookahead (steps)
KINDS = {"zh": 0, "rh": 1, "uh": 2, "zx": 3, "rx": 4, "ux": 5}


def dedup_ldweights(nc):
    """Remove back-to-back InstLdweights with identical physical APs (the
    PE keeps the stationary operand loaded across matmuls)."""
    removed = 0
    for f in nc.m.functions:
        for blk in f.blocks:
            out = []
            last = None
            for i in blk.instructions:
                tn = type(i).__name__
                if tn == "InstLdweights":
                    key = repr(i.ins[0])
                    if key == last and not (i.has_wait() or i.has_update()):
                        removed += 1
                        continue
                    last = key
                out.append(i)
            blk.instructions = out
    return removed


# ---------------------------------------------------------------- builder --
def build_program(nt=N // 128, T_steps=T):
    """Emit the per-core Bass program. nt = node tiles (N=nt*128)."""
    n = nt * 128
    HW = min(1024, n)
    halves = [(h, min(h + HW, n)) for h in range(0, n, HW)]

    def ch512(c0, c1):
        return [(a, min(a + 512, c1)) for a in range(c0, c1, 512)]

    nc = bacc.Bacc("TRN2", target_bir_lowering=False, debug=True)

    # -- external inputs (per core) --
    x_np = nc.dram_tensor("x_np", [T_steps, 128, nt, 128], BF16, kind="ExternalInput")
    x_cp = nc.dram_tensor("x_cp", [T_steps, 128, n], BF16, kind="ExternalInput")
    x_f32 = nc.dram_tensor("x_f32", [T_steps, 128, nt, 128], F32, kind="ExternalInput")
    S1 = nc.dram_tensor("S1", [128, nt, n], BF16, kind="ExternalInput")
    S2 = nc.dram_tensor("S2", [128, nt, n], BF16, kind="ExternalInput")
    wts = nc.dram_tensor("wts", [128, L * 6 * K, 128], BF16, kind="ExternalInput")
    biases = nc.dram_tensor("biases", [128, L * 3], F32, kind="ExternalInput")
    ident = nc.dram_tensor("ident", [128, 128], BF16, kind="ExternalInput")
    init_cp = nc.dram_tensor("init_cp", [L, 128, n], BF16, kind="ExternalInput")
    init_np = nc.dram_tensor("init_np", [L, 128, nt * 128], BF16, kind="ExternalInput")

    # -- external outputs --
    out_cur = nc.dram_tensor("out_cur", [T_steps, 128, nt, 128], F32, kind="ExternalOutput")
    out_hid = nc.dram_tensor("out_hid", [L, 128, n], F32, kind="ExternalOutput")

    def wslot(l, kind, k):
        return (l * 6 + KINDS[kind]) * K + k

    with tile.TileContext(nc) as tc:
        with (
            tc.tile_pool(name="const", bufs=1) as cpool,
            tc.tile_pool(name="state", bufs=1) as spool,
            tc.tile_pool(name="work", bufs=1) as wpool,
            tc.tile_pool(name="psum", bufs=1, space="PSUM") as ppool,
            tc.tile_pool(name="dram", bufs=1, space="DRAM") as dpool,
        ):
            # ---- persistent constants ----
            s1_sb = cpool.tile([128, nt, n], BF16, name="s1_sb")
            s2_sb = cpool.tile([128, nt, n], BF16, name="s2_sb")
            wts_sb = cpool.tile([128, L * 6 * K, 128], BF16, name="wts_sb")
            nc.sync.dma_start(wts_sb[:], wts[:])
            bias_sb = cpool.tile([128, L * 3], F32, name="bias_sb")
            nc.sync.dma_start(bias_sb[:], biases[:])
            id_sb = cpool.tile([128, 128], BF16, name="id_sb")
            nc.sync.dma_start(id_sb[:], ident[:])
            for jt in range(nt):
                nc.sync.dma_start(s1_sb[:, jt, :], S1[:, jt, :])
                nc.sync.dma_start(s2_sb[:, jt, :], S2[:, jt, :])

            # ---- DRAM scratch ----
            Gz_d = dpool.tile([L, T_steps, 128, n], BF16, name="Gz_d")
            Gr_d = dpool.tile([L, T_steps, 128, n], BF16, name="Gr_d")
            Uu_d = dpool.tile([L, T_steps, 128, n], BF16, name="Uu_d")
            seq_np_d = dpool.tile([T_steps, 128, nt * 128], BF16, name="seq_np_d")
            seq_cp_d = dpool.tile([T_steps, 128, n], BF16, name="seq_cp_d")

            def wt(name):
                return wpool.tile([128, n], BF16, tag="bigbf", bufs=8, name=name)

            def npt(name):
                return wpool.tile([128, nt, 128], BF16, tag="npbf", bufs=4, name=name)

            def hft(name, w):
                return wpool.tile([128, w], BF16, tag="hfbf", bufs=3, name=name)

            def w_ap(l, kind, k):
                sl = wslot(l, kind, k)
                return wts_sb[:, sl, :]

            def bias_ap(l, j):
                return bias_sb[:, l * 3 + j:l * 3 + j + 1]

            def seq_apply(stat_np, u1, u2):
                """u1 <- (A v)^T, u2 <- (A^2 v)^T (bf16 SBUF, CP layout)."""
                for (c0, c1) in halves:
                    p1 = ppool.tile([128, c1 - c0], F32, tag="psS", bufs=3,
                                    name="p1")
                    p2 = ppool.tile([128, c1 - c0], F32, tag="psS", bufs=3,
                                    name="p2")
                    for jt in range(nt):
                        st, sp = jt == 0, jt == nt - 1
                        lhs = stat_np[:, jt, :]
                        for (a0, a1) in ch512(c0, c1):
                            nc.tensor.matmul(p1[:, a0 - c0:a1 - c0], lhs,
                                             s1_sb[:, jt, a0:a1], start=st,
                                             stop=sp, skip_group_check=True)
                        for (a0, a1) in ch512(c0, c1):
                            nc.tensor.matmul(p2[:, a0 - c0:a1 - c0], lhs,
                                             s2_sb[:, jt, a0:a1], start=st,
                                             stop=sp, skip_group_check=True)
                    nc.vector.tensor_copy(u1[:, c0:c1], p1[:])
                    nc.scalar.activation(u2[:, c0:c1], p2[:], AF.Copy)

            def proj(l, kind, src3, gsb, dst, fn):
                """dst = fn(gsb + sum_k W[l,kind,k].T @ src3[k]) per batch."""
                for (c0, c1) in halves:
                    p = ppool.tile([128, c1 - c0], F32, tag="psS", bufs=3,
                                   name=f"p_{kind}")
                    for (a0, a1) in ch512(c0, c1):
                        nc.tensor.matmul(p[:, a0 - c0:a1 - c0], id_sb[:],
                                         gsb[:, a0:a1], start=True, stop=False,
                                         skip_group_check=True)
                    for k in range(K):
                        for (a0, a1) in ch512(c0, c1):
                            nc.tensor.matmul(
                                p[:, a0 - c0:a1 - c0], w_ap(l, kind, k),
                                src3[k][:, a0:a1], start=False,
                                stop=(k == K - 1), skip_group_check=True)
                    nc.scalar.activation(dst[:, c0:c1], p[:], fn)

            def pre_apply(stat_np, y1, y2):
                """Precompute-stream A applies through the single psP slot."""
                for (Ssb, dst, eng) in ((s1_sb, y1, 0), (s2_sb, y2, 1)):
                    for (c0, c1) in halves:
                        p = ppool.tile([128, c1 - c0], F32, tag="psP", bufs=1,
                                       name="pa")
                        for jt in range(nt):
                            st, sp = jt == 0, jt == nt - 1
                            for (a0, a1) in ch512(c0, c1):
                                nc.tensor.matmul(p[:, a0 - c0:a1 - c0],
                                                 stat_np[:, jt, :],
                                                 Ssb[:, jt, a0:a1], start=st,
                                                 stop=sp, skip_group_check=True)
                        if eng == 0:
                            nc.vector.tensor_copy(dst[:, c0:c1], p[:])
                        else:
                            nc.scalar.activation(dst[:, c0:c1], p[:], AF.Copy)

            def pre_proj(l, kind, src3, bias_j, dst_dram, t):
                for (c0, c1) in halves:
                    p = ppool.tile([128, c1 - c0], F32, tag="psP", bufs=1,
                                   name=f"pp_{kind}")
                    for k in range(K):
                        for (a0, a1) in ch512(c0, c1):
                            nc.tensor.matmul(
                                p[:, a0 - c0:a1 - c0], w_ap(l, kind, k),
                                src3[k][:, a0:a1], start=(k == 0),
                                stop=(k == K - 1), skip_group_check=True)
                    ge = hft(f"ge_{kind}", c1 - c0)
                    nc.scalar.activation(ge[:], p[:], AF.Identity,
                                         bias=bias_ap(l, bias_j))
                    nc.sync.dma_start(dst_dram[l, t][:, c0:c1], ge[:])

            def cp_to_np_half(src_cp, dst_np, c0, c1):
                pt = ppool.tile([128, c1 - c0], BF16, tag="psS", bufs=3,
                                name="pt")
                for j, jt in enumerate(range(c0 // 128, c1 // 128)):
                    sl = slice(jt * 128, (jt + 1) * 128)
                    nc.tensor.transpose(pt[:, j * 128:(j + 1) * 128],
                                        src_cp[:, sl], id_sb[:])
                nc.vector.tensor_copy(
                    dst_np.rearrange("p a b -> p (a b)")[:, c0:c1], pt[:])

            def emit_pre(l, t):
                xnp = npt("xnp")
                nc.sync.dma_start(xnp[:], x_np[t])
                xcp = wt("xcp")
                nc.sync.dma_start(xcp[:], x_cp[t])
                if l == 1:
                    sa = npt("sa")
                    nc.sync.dma_start(sa[:], seq_np_d[t])
                    nc.vector.tensor_add(xnp.rearrange("p a b -> p (a b)"),
                                         xnp.rearrange("p a b -> p (a b)"),
                                         sa.rearrange("p a b -> p (a b)"))
                    sb_ = wt("sb_")
                    nc.sync.dma_start(sb_[:], seq_cp_d[t])
                    nc.vector.tensor_add(xcp[:], xcp[:], sb_[:])
                y1 = wt("y1")
                y2 = wt("y2")
                pre_apply(xnp, y1, y2)
                src3 = [xcp, y1, y2]
                pre_proj(l, "zx", src3, 0, Gz_d, t)
                pre_proj(l, "rx", src3, 1, Gr_d, t)
                pre_proj(l, "ux", src3, 2, Uu_d, t)

            def emit_step(l, t, h_cp, h_np, last):
                gz = wt("gz")
                nc.sync.dma_start(gz[:], Gz_d[l, t])
                gr = wt("gr")
                nc.sync.dma_start(gr[:], Gr_d[l, t])
                uu = wt("uu")
                nc.sync.dma_start(uu[:], Uu_d[l, t])

                u1 = wt("u1")
                u2 = wt("u2")
                seq_apply(h_np, u1, u2)
                z = wt("z")
                proj(l, "zh", [h_cp, u1, u2], gz, z, AF.Sigmoid)
                r = wt("r")
                proj(l, "rh", [h_cp, u1, u2], gr, r, AF.Sigmoid)

                cc = wt("cc")
                for (c0, c1) in halves:
                    nc.vector.tensor_mul(cc[:, c0:c1], z[:, c0:c1],
                                         h_cp[:, c0:c1])
                c_np = npt("c_np")
                for (c0, c1) in halves:
                    cp_to_np_half(cc, c_np, c0, c1)

                v1 = wt("v1")
                v2 = wt("v2")
                seq_apply(c_np, v1, v2)
                hc = wt("hc")
                proj(l, "uh", [cc, v1, v2], uu, hc, AF.Tanh)

                # h_new = hc + r * (h - hc)
                h_cp_new = spool.tile([128, n], BF16, tag="hcp", bufs=2,
                                      name="hcpn")
                for (c0, c1) in halves:
                    d1 = hft("d1", c1 - c0)
                    nc.vector.tensor_sub(d1[:], h_cp[:, c0:c1], hc[:, c0:c1])
                    nc.vector.tensor_mul(d1[:], r[:, c0:c1], d1[:])
                    nc.vector.tensor_add(h_cp_new[:, c0:c1], hc[:, c0:c1],
                                         d1[:])
                h_np_new = npt("h_npn")
                for (c0, c1) in halves:
                    cp_to_np_half(h_cp_new, h_np_new, c0, c1)

                if l == 0:
                    nc.sync.dma_start(seq_cp_d[t], h_cp_new[:])
                    nc.sync.dma_start(seq_np_d[t],
                                      h_np_new.rearrange("p a b -> p (a b)"))
                else:
                    nh = max(1, nt // 2)
                    for hh in range(nt // nh):
                        hsl = slice(hh * nh, (hh + 1) * nh)
                        fl = slice(hh * nh * 128, (hh + 1) * nh * 128)
                        xf = wpool.tile([128, nh, 128], F32, tag="f32h",
                                        bufs=2, name="xf")
                        nc.sync.dma_start(xf[:], x_f32[t, :, hsl, :])
                        ob = wpool.tile([128, nh, 128], F32, tag="f32h",
                                        bufs=2, name="ob")
                        nc.vector.tensor_add(
                            ob.rearrange("p a b -> p (a b)"),
                            xf.rearrange("p a b -> p (a b)"),
                            h_np_new.rearrange("p a b -> p (a b)")[:, fl])
                        nc.sync.dma_start(out_cur[t, :, hsl, :], ob[:])

                if last:
                    for (c0, c1) in halves:
                        hl = wpool.tile([128, c1 - c0], F32, tag="f32h",
                                        bufs=2, name="hl")
                        nc.vector.tensor_copy(hl[:], h_cp_new[:, c0:c1])
                        nc.sync.dma_start(out_hid[l][:, c0:c1], hl[:])

                return h_cp_new, h_np_new

            # ================= per-layer phases =================
            for l in range(L):
                npre = min(PRE, T_steps)
                for t in range(npre):
                    emit_pre(l, t)
                h_cp = spool.tile([128, n], BF16, tag="hcp", bufs=2,
                                  name="hcp0")
                nc.sync.dma_start(h_cp[:], init_cp[l])
                h_np = npt("hnp0")
                nc.sync.dma_start(h_np.rearrange("p a b -> p (a b)"),
                                  init_np[l])
                for t in range(T_steps):
                    h_cp, h_np = emit_step(l, t, h_cp, h_np,
                                           t == T_steps - 1)
                    if t + npre < T_steps:
                        emit_pre(l, t + npre)

    nc.compile()
    ndup = dedup_ldweights(nc)
    print(f"[kernel] dedup removed {ndup} LDWEIGHTS", file=sys.stderr)
    return nc


# ------------------------------------------------------------- host side --
def fold_cheb(W):
    """[K, cin, cout] Chebyshev -> power-basis weights."""
    return np.stack([W[0] - W[2], W[1], 2.0 * W[2]], axis=0)


def prep_shared(adj, Wg, bg, Wu, bu, nt=N // 128):
    n = nt * 128
    A = np.asarray(adj, np.float32)
    A2 = (A @ A).astype(np.float32)
    S1h = np.ascontiguousarray(
        A.T.reshape(nt, 128, n).transpose(1, 0, 2)).astype(bf16)
    S2h = np.ascontiguousarray(
        A2.T.reshape(nt, 128, n).transpose(1, 0, 2)).astype(bf16)

    Wg = np.asarray(Wg, np.float32)
    Wu = np.asarray(Wu, np.float32)
    wts = np.zeros((128, L * 6 * K, 128), np.float32)
    for l in range(L):
        Wgf, Wuf = fold_cheb(Wg[l]), fold_cheb(Wu[l])
        for k in range(K):
            sl = {
                "zh": Wgf[k][D:, :D], "rh": Wgf[k][D:, D:], "uh": Wuf[k][D:, :],
                "zx": Wgf[k][:D, :D], "rx": Wgf[k][:D, D:], "ux": Wuf[k][:D, :],
            }
            for kind, w in sl.items():
                j = (l * 6 + KINDS[kind]) * K + k
                wts[0:64, j, 0:64] = w
                wts[64:128, j, 64:128] = w
    wtsh = wts.astype(bf16)

    bg = np.asarray(bg, np.float32)
    bu = np.asarray(bu, np.float32)
    biases = np.zeros((128, L * 3), np.float32)
    for l in range(L):
        biases[:, l * 3 + 0] = np.tile(bg[l][:D], 2)
        biases[:, l * 3 + 1] = np.tile(bg[l][D:], 2)
        biases[:, l * 3 + 2] = np.tile(bu[l], 2)

    identh = np.eye(128, dtype=np.float32).astype(bf16)
    return dict(S1=S1h, S2=S2h, wts=wtsh, biases=biases, ident=identh)


def prep_core(xc, init_c, shared, nt=N // 128, T_steps=T):
    """xc: [BL, T_steps, n, D] f32; init_c: [L, BL, n, D] f32."""
    n = nt * 128
    xc = np.asarray(xc, np.float32)
    # NP: [t, p, jt, (b, d)]
    xnp = np.ascontiguousarray(
        xc.reshape(BL, T_steps, nt, 128, D).transpose(1, 3, 2, 0, 4)
        .reshape(T_steps, 128, nt, 128))
    # CP: [t, (b, d), node]
    xcp = np.ascontiguousarray(
        xc.transpose(1, 0, 3, 2).reshape(T_steps, 128, n))
    ic = np.asarray(init_c, np.float32)
    init_cp_h = np.ascontiguousarray(
        ic.transpose(0, 1, 3, 2).reshape(L, 128, n)).astype(bf16)
    init_np_h = np.ascontiguousarray(
        ic.reshape(L, BL, nt, 128, D).transpose(0, 3, 2, 1, 4)
        .reshape(L, 128, nt * 128)).astype(bf16)
    m = dict(shared)
    m.update(
        x_np=xnp.astype(bf16),
        x_cp=xcp.astype(bf16),
        x_f32=xnp,
        init_cp=init_cp_h,
        init_np=init_np_h,
    )
    return m


def decode_outputs(results, nt=N // 128, T_steps=T):
    """results: list per core of {out_cur, out_hid} -> (current, hiddens)."""
    n = nt * 128
    curs, hids = [], []
    for res in results:
        oc = np.asarray(res["out_cur"], np.float32)  # [t, p, jt, (b,d)]
        cur = (oc.reshape(T_steps, 128, nt, BL, D)
               .transpose(3, 0, 2, 1, 4).reshape(BL, T_steps, n, D))
        curs.append(cur)
        oh = np.asarray(res["out_hid"], np.float32)  # [L, (b,ch), node]
        hid = oh.reshape(L, BL, D, n).transpose(0, 1, 3, 2)  # [L, BL, n, D]
        hids.append(hid)
    current = np.concatenate(curs, axis=0)           # [B, T, N, D]
    hiddens = np.concatenate(hids, axis=1)           # [L, B, N, D]
    return current, hiddens


_NC_CACHE = {}


def get_program(nt=N // 128, T_steps=T):
    key = (nt, T_steps)
    if key not in _NC_CACHE:
        _NC_CACHE[key] = build_program(nt, T_steps)
    return _NC_CACHE[key]


def make_in_maps(x, init_state, adj_matrix, Wg, bg, Wu, bu,
                 nt=N // 128, T_steps=T):
    shared = prep_shared(adj_matrix, Wg, bg, Wu, bu, nt)
    x = np.asarray(x, np.float32)
    ist = np.asarray(init_state, np.float32)
    in_maps = []
    for c in range(N_CORES):
        xc = x[c * BL:(c + 1) * BL]
        ic = ist[:, c * BL:(c + 1) * BL]
        in_maps.append(prep_core(xc, ic, shared, nt, T_steps))
    return in_maps


def kernel(x, init_state, adj_matrix, Wg, bg, Wu, bu):
    nc = get_program()
    in_maps = make_in_maps(x, init_state, adj_matrix, Wg, bg, Wu, bu)
    res = run_bass_kernel_spmd(nc, in_maps, core_ids=list(range(N_CORES)))
    return decode_outputs(res.results)


# optional: traced run for profiling (used by test.py)
def run_traced(x, init_state, adj_matrix, Wg, bg, Wu, bu, tmpdir=None):
    import antenv
    from trn_agent_boot.trn_boot import _ntff_profile_via_ctypes
    hook = _ntff_profile_via_ctypes('/opt/axon/libaxon_pjrt.so')
    mod = types.ModuleType('antenv.axon_hooks')
    mod.get_axon_ntff_profile_hook = lambda: hook
    sys.modules['antenv.axon_hooks'] = mod
    antenv.axon_hooks = mod
    nc = get_program()
    in_maps = make_in_maps(x, init_state, adj_matrix, Wg, bg, Wu, bu)
    res = run_bass_kernel_spmd(nc, in_maps, core_ids=list(range(N_CORES)),
                               trace=True, tmpdir=tmpdir)
    return decode_outputs(res.results), res


# revision 13
# speedup vs baseline: 2.2026x; 1.2006x over previous
"""Trainium2 Bass kernel for nn_AVWDCRNN (2-layer Chebyshev graph-conv GRU).

Strategy (per spec sharding hint): data-parallel over batch B=16 across 8
cores (2 batch elements per core), adjacency/weights replicated; the time
recurrence runs sequentially on-chip.

Algebraic restructuring:
  - Chebyshev supports {I, A, 2A^2-I} folded into powers {I, A, A^2} with
    host-folded weights W'0=W0-W2, W'1=W1, W'2=2*W2.
  - A^T and (A^2)^T stay SBUF-resident in bf16; per-step state matmuls use
    (stationary = state-tile node-partitioned, moving = A^T) so both A and
    A^2 applications share stationary loads and come out channel-partitioned
    ready for the (small) weight projections.
  - x-dependent gate/candidate contributions are precomputed per layer as
    G/U tensors; the precompute for step t+PRE is emitted interleaved with
    sequential step t so its matmuls fill PE gaps (keeps HAM warm).
  - Node dim processed in 1024-column halves so PSUM turns over quickly and
    evictions/projections pipeline against the next half's matmuls.
  - Redundant LDWEIGHTS (same stationary reloaded back-to-back) are removed
    by a post-compile pass.

Self-contained: hardcodes shapes; only imports the system concourse stack.
"""
import sys
import types

for _p in ("/opt/trn_rl_repo",):
    if _p not in sys.path:
        sys.path.insert(0, _p)

import numpy as np
import ml_dtypes

import concourse.bacc as bacc
import concourse.bass as bass
import concourse.mybir as mybir
import concourse.tile as tile
from concourse.bass_utils import run_bass_kernel_spmd

BF16 = mybir.dt.bfloat16
F32 = mybir.dt.float32
F8 = mybir.dt.float8e4
AF = mybir.ActivationFunctionType
bf16 = ml_dtypes.bfloat16
fp8 = ml_dtypes.float8_e4m3
SC = 1024.0                # fp8 support-matrix scale (folded into weights)

# problem constants
B, T, N, D = 16, 24, 2048, 64
L, K = 2, 3
N_CORES = 8
BL = B // N_CORES          # batch per core
BD = BL * D                # 128: (b, d) packed columns
PRE = 3                    # precompute lookahead (steps)
KINDS = {"zh": 0, "rh": 1, "uh": 2, "zx": 3, "rx": 4, "ux": 5}


def dedup_ldweights(nc):
    """Remove back-to-back InstLdweights with identical physical APs (the
    PE keeps the stationary operand loaded across matmuls)."""
    removed = 0
    for f in nc.m.functions:
        for blk in f.blocks:
            out = []
            last = None
            for i in blk.instructions:
                tn = type(i).__name__
                if tn == "InstLdweights":
                    key = repr(i.ins[0])
                    if key == last and not (i.has_wait() or i.has_update()):
                        removed += 1
                        continue
                    last = key
                out.append(i)
            blk.instructions = out
    return removed


# ---------------------------------------------------------------- builder --
def build_program(nt=N // 128, T_steps=T):
    """Emit the per-core Bass program. nt = node tiles (N=nt*128)."""
    n = nt * 128
    HW = min(1024, n)
    halves = [(h, min(h + HW, n)) for h in range(0, n, HW)]

    def ch512(c0, c1):
        return [(a, min(a + 512, c1)) for a in range(c0, c1, 512)]

    nc = bacc.Bacc("TRN2", target_bir_lowering=False, debug=True)

    # -- external inputs (per core) --
    x_np = nc.dram_tensor("x_np", [T_steps, 128, nt, 128], F8, kind="ExternalInput")
    x_cp = nc.dram_tensor("x_cp", [T_steps, 128, n], BF16, kind="ExternalInput")
    x_f32 = nc.dram_tensor("x_f32", [T_steps, 128, nt, 128], F32, kind="ExternalInput")
    S1 = nc.dram_tensor("S1", [128, nt, n], F8, kind="ExternalInput")
    S2 = nc.dram_tensor("S2", [128, nt, n], F8, kind="ExternalInput")
    wts = nc.dram_tensor("wts", [128, L * 6 * K, 128], BF16, kind="ExternalInput")
    biases = nc.dram_tensor("biases", [128, L * 3], F32, kind="ExternalInput")
    ident = nc.dram_tensor("ident", [128, 128], BF16, kind="ExternalInput")
    init_cp = nc.dram_tensor("init_cp", [L, 128, n], F32, kind="ExternalInput")
    init_np = nc.dram_tensor("init_np", [L, 128, nt * 128], F8, kind="ExternalInput")

    # -- external outputs --
    out_cur = nc.dram_tensor("out_cur", [T_steps, 128, nt, 128], F32, kind="ExternalOutput")
    out_hid = nc.dram_tensor("out_hid", [L, 128, n], F32, kind="ExternalOutput")

    def wslot(l, kind, k):
        return (l * 6 + KINDS[kind]) * K + k

    with tile.TileContext(nc) as tc:
        with (
            tc.tile_pool(name="const", bufs=1) as cpool,
            tc.tile_pool(name="state", bufs=1) as spool,
            tc.tile_pool(name="work", bufs=1) as wpool,
            tc.tile_pool(name="psum", bufs=1, space="PSUM") as ppool,
            tc.tile_pool(name="dram", bufs=1, space="DRAM") as dpool,
        ):
            # ---- persistent constants ----
            s1_sb = cpool.tile([128, nt, n], F8, name="s1_sb")
            s2_sb = cpool.tile([128, nt, n], F8, name="s2_sb")
            wts_sb = cpool.tile([128, L * 6 * K, 128], BF16, name="wts_sb")
            nc.sync.dma_start(wts_sb[:], wts[:])
            bias_sb = cpool.tile([128, L * 3], F32, name="bias_sb")
            nc.sync.dma_start(bias_sb[:], biases[:])
            id_sb = cpool.tile([128, 128], BF16, name="id_sb")
            nc.sync.dma_start(id_sb[:], ident[:])
            for jt in range(nt):
                nc.sync.dma_start(s1_sb[:, jt, :], S1[:, jt, :])
                nc.sync.dma_start(s2_sb[:, jt, :], S2[:, jt, :])

            # ---- DRAM scratch ----
            Gz_d = dpool.tile([L, T_steps, 128, n], BF16, name="Gz_d")
            Gr_d = dpool.tile([L, T_steps, 128, n], BF16, name="Gr_d")
            Uu_d = dpool.tile([L, T_steps, 128, n], BF16, name="Uu_d")
            seq_np_d = dpool.tile([T_steps, 128, nt * 128], F8, name="seq_np_d")
            seq_cp_d = dpool.tile([T_steps, 128, n], BF16, name="seq_cp_d")

            def wt(name):
                return wpool.tile([128, n], BF16, tag="bigbf", bufs=10, name=name)

            def npt(name):
                return wpool.tile([128, nt, 128], F8, tag="npbf", bufs=6, name=name)

            def npbt(name):
                return wpool.tile([128, nt, 128], BF16, tag="npb", bufs=2, name=name)

            def hft(name, w):
                return wpool.tile([128, w], BF16, tag="hfbf", bufs=3, name=name)

            def f32t(name, w):
                return wpool.tile([128, w], F32, tag="f32h", bufs=4, name=name)

            def w_ap(l, kind, k):
                sl = wslot(l, kind, k)
                return wts_sb[:, sl, :]

            def bias_ap(l, j):
                return bias_sb[:, l * 3 + j:l * 3 + j + 1]

            def seq_apply(stat_np, u1, u2):
                """u1 <- (A v)^T, u2 <- (A^2 v)^T (bf16 SBUF, CP layout)."""
                for (c0, c1) in halves:
                    p1 = ppool.tile([128, c1 - c0], F32, tag="psS", bufs=3,
                                    name="p1")
                    p2 = ppool.tile([128, c1 - c0], F32, tag="psS", bufs=3,
                                    name="p2")
                    for jt in range(0, nt, 2):
                        st, sp = jt == 0, jt == nt - 2
                        lhs = stat_np[:, jt:jt + 2, :]
                        for (a0, a1) in ch512(c0, c1):
                            nc.tensor.matmul(p1[:, a0 - c0:a1 - c0], lhs,
                                             s1_sb[:, jt:jt + 2, a0:a1], start=st,
                                             stop=sp, skip_group_check=True,
                                             perf_mode=mybir.MatmulPerfMode.DoubleRow)
                        for (a0, a1) in ch512(c0, c1):
                            nc.tensor.matmul(p2[:, a0 - c0:a1 - c0], lhs,
                                             s2_sb[:, jt:jt + 2, a0:a1], start=st,
                                             stop=sp, skip_group_check=True,
                                             perf_mode=mybir.MatmulPerfMode.DoubleRow)
                    nc.vector.tensor_copy(u1[:, c0:c1], p1[:])
                    nc.scalar.activation(u2[:, c0:c1], p2[:], AF.Copy)

            def proj(l, kind, src3, gsb, dst, fn):
                """dst = fn(gsb + sum_k W[l,kind,k].T @ src3[k]) per batch."""
                for (c0, c1) in halves:
                    p = ppool.tile([128, c1 - c0], F32, tag="psS", bufs=3,
                                   name=f"p_{kind}")
                    for (a0, a1) in ch512(c0, c1):
                        nc.tensor.matmul(p[:, a0 - c0:a1 - c0], id_sb[:],
                                         gsb[:, a0:a1], start=True, stop=False,
                                         skip_group_check=True)
                    for k in range(K):
                        for (a0, a1) in ch512(c0, c1):
                            nc.tensor.matmul(
                                p[:, a0 - c0:a1 - c0], w_ap(l, kind, k),
                                src3[k][:, a0:a1], start=False,
                                stop=(k == K - 1), skip_group_check=True)
                    nc.scalar.activation(dst[:, c0:c1], p[:], fn)

            def pre_apply(stat_np, y1, y2):
                """Precompute-stream A applies through the single psP slot."""
                for (Ssb, dst, eng) in ((s1_sb, y1, 0), (s2_sb, y2, 1)):
                    for (c0, c1) in halves:
                        p = ppool.tile([128, c1 - c0], F32, tag="psP", bufs=1,
                                       name="pa")
                        for jt in range(0, nt, 2):
                            st, sp = jt == 0, jt == nt - 2
                            for (a0, a1) in ch512(c0, c1):
                                nc.tensor.matmul(p[:, a0 - c0:a1 - c0],
                                                 stat_np[:, jt:jt + 2, :],
                                                 Ssb[:, jt:jt + 2, a0:a1], start=st,
                                                 stop=sp, skip_group_check=True,
                                                 perf_mode=mybir.MatmulPerfMode.DoubleRow)
                        if eng == 0:
                            nc.vector.tensor_copy(dst[:, c0:c1], p[:])
                        else:
                            nc.scalar.activation(dst[:, c0:c1], p[:], AF.Copy)

            def pre_proj(l, kind, src3, bias_j, dst_dram, t):
                for (c0, c1) in halves:
                    p = ppool.tile([128, c1 - c0], F32, tag="psP", bufs=1,
                                   name=f"pp_{kind}")
                    for k in range(K):
                        for (a0, a1) in ch512(c0, c1):
                            nc.tensor.matmul(
                                p[:, a0 - c0:a1 - c0], w_ap(l, kind, k),
                                src3[k][:, a0:a1], start=(k == 0),
                                stop=(k == K - 1), skip_group_check=True)
                    ge = hft(f"ge_{kind}", c1 - c0)
                    nc.scalar.activation(ge[:], p[:], AF.Identity,
                                         bias=bias_ap(l, bias_j))
                    nc.sync.dma_start(dst_dram[l, t][:, c0:c1], ge[:])

            def cp_to_np_half(src_cp, dst_np, c0, c1, dst_bf=None):
                pt = ppool.tile([128, c1 - c0], BF16, tag="psS", bufs=3,
                                name="pt")
                for j, jt in enumerate(range(c0 // 128, c1 // 128)):
                    sl = slice(jt * 128, (jt + 1) * 128)
                    nc.tensor.transpose(pt[:, j * 128:(j + 1) * 128],
                                        src_cp[:, sl], id_sb[:])
                nc.vector.tensor_copy(
                    dst_np.rearrange("p a b -> p (a b)")[:, c0:c1], pt[:])
                if dst_bf is not None:
                    nc.scalar.activation(
                        dst_bf.rearrange("p a b -> p (a b)")[:, c0:c1], pt[:],
                        AF.Copy)

            def emit_pre(l, t):
                xnp = npt("xnp")
                nc.sync.dma_start(xnp[:], x_np[t])
                xcp = wt("xcp")
                nc.sync.dma_start(xcp[:], x_cp[t])
                if l == 1:
                    sa = npt("sa")
                    nc.sync.dma_start(sa[:], seq_np_d[t])
                    nc.vector.tensor_add(xnp.rearrange("p a b -> p (a b)"),
                                         xnp.rearrange("p a b -> p (a b)"),
                                         sa.rearrange("p a b -> p (a b)"))
                    sb_ = wt("sb_")
                    nc.sync.dma_start(sb_[:], seq_cp_d[t])
                    nc.vector.tensor_add(xcp[:], xcp[:], sb_[:])
                y1 = wt("y1")
                y2 = wt("y2")
                pre_apply(xnp, y1, y2)
                src3 = [xcp, y1, y2]
                pre_proj(l, "zx", src3, 0, Gz_d, t)
                pre_proj(l, "rx", src3, 1, Gr_d, t)
                pre_proj(l, "ux", src3, 2, Uu_d, t)

            def emit_step(l, t, h_cp, h_bf, h_np, last):
                gz = wt("gz")
                nc.sync.dma_start(gz[:], Gz_d[l, t])
                gr = wt("gr")
                nc.sync.dma_start(gr[:], Gr_d[l, t])
                uu = wt("uu")
                nc.sync.dma_start(uu[:], Uu_d[l, t])

                u1 = wt("u1")
                u2 = wt("u2")
                seq_apply(h_np, u1, u2)
                z = wt("z")
                proj(l, "zh", [h_bf, u1, u2], gz, z, AF.Sigmoid)
                r = wt("r")
                proj(l, "rh", [h_bf, u1, u2], gr, r, AF.Sigmoid)

                cc = wt("cc")
                for (c0, c1) in halves:
                    nc.vector.tensor_mul(cc[:, c0:c1], z[:, c0:c1],
                                         h_cp[:, c0:c1])
                c_np = npt("c_np")
                for (c0, c1) in halves:
                    cp_to_np_half(cc, c_np, c0, c1)

                v1 = wt("v1")
                v2 = wt("v2")
                seq_apply(c_np, v1, v2)
                hc = wt("hc")
                proj(l, "uh", [cc, v1, v2], uu, hc, AF.Tanh)

                # h_new = hc + r * (h - hc)  (f32 state)
                h_cp_new = spool.tile([128, n], F32, tag="hcp", bufs=2,
                                      name="hcpn")
                h_bf_new = wt("h_bfn")
                for (c0, c1) in halves:
                    d1 = f32t("d1", c1 - c0)
                    nc.vector.tensor_sub(d1[:], h_cp[:, c0:c1], hc[:, c0:c1])
                    nc.vector.tensor_mul(d1[:], r[:, c0:c1], d1[:])
                    nc.vector.tensor_add(h_cp_new[:, c0:c1], hc[:, c0:c1],
                                         d1[:])
                    nc.vector.tensor_copy(h_bf_new[:, c0:c1],
                                          h_cp_new[:, c0:c1])
                h_np_new = npt("h_npn")
                h_npb = npbt("h_npb") if l == 1 else None
                for (c0, c1) in halves:
                    cp_to_np_half(h_bf_new, h_np_new, c0, c1, dst_bf=h_npb)

                if l == 0:
                    nc.sync.dma_start(seq_cp_d[t], h_bf_new[:])
                    nc.sync.dma_start(seq_np_d[t],
                                      h_np_new.rearrange("p a b -> p (a b)"))
                else:
                    nh = max(1, nt // 2)
                    for hh in range(nt // nh):
                        hsl = slice(hh * nh, (hh + 1) * nh)
                        fl = slice(hh * nh * 128, (hh + 1) * nh * 128)
                        xf = wpool.tile([128, nh, 128], F32, tag="f32h",
                                        bufs=4, name="xf")
                        nc.sync.dma_start(xf[:], x_f32[t, :, hsl, :])
                        ob = wpool.tile([128, nh, 128], F32, tag="f32h",
                                        bufs=4, name="ob")
                        nc.vector.tensor_add(
                            ob.rearrange("p a b -> p (a b)"),
                            xf.rearrange("p a b -> p (a b)"),
                            h_npb.rearrange("p a b -> p (a b)")[:, fl])
                        nc.sync.dma_start(out_cur[t, :, hsl, :], ob[:])

                if last:
                    for (c0, c1) in halves:
                        hl = wpool.tile([128, c1 - c0], F32, tag="f32h",
                                        bufs=4, name="hl")
                        nc.vector.tensor_copy(hl[:], h_cp_new[:, c0:c1])
                        nc.sync.dma_start(out_hid[l][:, c0:c1], hl[:])

                return h_cp_new, h_bf_new, h_np_new

            # ================= per-layer phases =================
            for l in range(L):
                npre = min(PRE, T_steps)
                for t in range(npre):
                    emit_pre(l, t)
                h_cp = spool.tile([128, n], F32, tag="hcp", bufs=2,
                                  name="hcp0")
                nc.sync.dma_start(h_cp[:], init_cp[l])
                h_bf = wt("hbf0")
                nc.vector.tensor_copy(h_bf[:], h_cp[:])
                h_np = npt("hnp0")
                nc.sync.dma_start(h_np.rearrange("p a b -> p (a b)"),
                                  init_np[l])
                for t in range(T_steps):
                    h_cp, h_bf, h_np = emit_step(l, t, h_cp, h_bf, h_np,
                                                 t == T_steps - 1)
                    if t + npre < T_steps:
                        emit_pre(l, t + npre)

    nc.compile()
    ndup = dedup_ldweights(nc)
    print(f"[kernel] dedup removed {ndup} LDWEIGHTS", file=sys.stderr)
    return nc


# ------------------------------------------------------------- host side --
def fold_cheb(W):
    """[K, cin, cout] Chebyshev -> power-basis weights."""
    return np.stack([W[0] - W[2], W[1], 2.0 * W[2]], axis=0)


def prep_shared(adj, Wg, bg, Wu, bu, nt=N // 128):
    n = nt * 128
    A = np.asarray(adj, np.float32)
    A2 = (A @ A).astype(np.float32)
    S1h = np.ascontiguousarray(
        (A.T * SC).reshape(nt, 128, n).transpose(1, 0, 2)).astype(fp8)
    S2h = np.ascontiguousarray(
        (A2.T * SC).reshape(nt, 128, n).transpose(1, 0, 2)).astype(fp8)

    Wg = np.asarray(Wg, np.float32)
    Wu = np.asarray(Wu, np.float32)
    wts = np.zeros((128, L * 6 * K, 128), np.float32)
    for l in range(L):
        Wgf, Wuf = fold_cheb(Wg[l]), fold_cheb(Wu[l])
        Wgf[1] /= SC
        Wgf[2] /= SC
        Wuf[1] /= SC
        Wuf[2] /= SC
        for k in range(K):
            sl = {
                "zh": Wgf[k][D:, :D], "rh": Wgf[k][D:, D:], "uh": Wuf[k][D:, :],
                "zx": Wgf[k][:D, :D], "rx": Wgf[k][:D, D:], "ux": Wuf[k][:D, :],
            }
            for kind, w in sl.items():
                j = (l * 6 + KINDS[kind]) * K + k
                wts[0:64, j, 0:64] = w
                wts[64:128, j, 64:128] = w
    wtsh = wts.astype(bf16)

    bg = np.asarray(bg, np.float32)
    bu = np.asarray(bu, np.float32)
    biases = np.zeros((128, L * 3), np.float32)
    for l in range(L):
        biases[:, l * 3 + 0] = np.tile(bg[l][:D], 2)
        biases[:, l * 3 + 1] = np.tile(bg[l][D:], 2)
        biases[:, l * 3 + 2] = np.tile(bu[l], 2)

    identh = np.eye(128, dtype=np.float32).astype(bf16)
    return dict(S1=S1h, S2=S2h, wts=wtsh, biases=biases, ident=identh)


def prep_core(xc, init_c, shared, nt=N // 128, T_steps=T):
    """xc: [BL, T_steps, n, D] f32; init_c: [L, BL, n, D] f32."""
    n = nt * 128
    xc = np.asarray(xc, np.float32)
    # NP: [t, p, jt, (b, d)]
    xnp = np.ascontiguousarray(
        xc.reshape(BL, T_steps, nt, 128, D).transpose(1, 3, 2, 0, 4)
        .reshape(T_steps, 128, nt, 128))
    # CP: [t, (b, d), node]
    xcp = np.ascontiguousarray(
        xc.transpose(1, 0, 3, 2).reshape(T_steps, 128, n))
    ic = np.asarray(init_c, np.float32)
    init_cp_h = np.ascontiguousarray(
        ic.transpose(0, 1, 3, 2).reshape(L, 128, n))
    init_np_h = np.ascontiguousarray(
        ic.reshape(L, BL, nt, 128, D).transpose(0, 3, 2, 1, 4)
        .reshape(L, 128, nt * 128)).astype(fp8)
    m = dict(shared)
    m.update(
        x_np=xnp.astype(fp8),
        x_cp=xcp.astype(bf16),
        x_f32=xnp,
        init_cp=init_cp_h,
        init_np=init_np_h,
    )
    return m


def decode_outputs(results, nt=N // 128, T_steps=T):
    """results: list per core of {out_cur, out_hid} -> (current, hiddens)."""
    n = nt * 128
    curs, hids = [], []
    for res in results:
        oc = np.asarray(res["out_cur"], np.float32)  # [t, p, jt, (b,d)]
        cur = (oc.reshape(T_steps, 128, nt, BL, D)
               .transpose(3, 0, 2, 1, 4).reshape(BL, T_steps, n, D))
        curs.append(cur)
        oh = np.asarray(res["out_hid"], np.float32)  # [L, (b,ch), node]
        hid = oh.reshape(L, BL, D, n).transpose(0, 1, 3, 2)  # [L, BL, n, D]
        hids.append(hid)
    current = np.concatenate(curs, axis=0)           # [B, T, N, D]
    hiddens = np.concatenate(hids, axis=1)           # [L, B, N, D]
    return current, hiddens


_NC_CACHE = {}


def get_program(nt=N // 128, T_steps=T):
    key = (nt, T_steps)
    if key not in _NC_CACHE:
        _NC_CACHE[key] = build_program(nt, T_steps)
    return _NC_CACHE[key]


def make_in_maps(x, init_state, adj_matrix, Wg, bg, Wu, bu,
                 nt=N // 128, T_steps=T):
    shared = prep_shared(adj_matrix, Wg, bg, Wu, bu, nt)
    x = np.asarray(x, np.float32)
    ist = np.asarray(init_state, np.float32)
    in_maps = []
    for c in range(N_CORES):
        xc = x[c * BL:(c + 1) * BL]
        ic = ist[:, c * BL:(c + 1) * BL]
        in_maps.append(prep_core(xc, ic, shared, nt, T_steps))
    return in_maps


def kernel(x, init_state, adj_matrix, Wg, bg, Wu, bu):
    nc = get_program()
    in_maps = make_in_maps(x, init_state, adj_matrix, Wg, bg, Wu, bu)
    res = run_bass_kernel_spmd(nc, in_maps, core_ids=list(range(N_CORES)))
    return decode_outputs(res.results)


# optional: traced run for profiling (used by test.py)
def run_traced(x, init_state, adj_matrix, Wg, bg, Wu, bu, tmpdir=None):
    import antenv
    from trn_agent_boot.trn_boot import _ntff_profile_via_ctypes
    hook = _ntff_profile_via_ctypes('/opt/axon/libaxon_pjrt.so')
    mod = types.ModuleType('antenv.axon_hooks')
    mod.get_axon_ntff_profile_hook = lambda: hook
    sys.modules['antenv.axon_hooks'] = mod
    antenv.axon_hooks = mod
    nc = get_program()
    in_maps = make_in_maps(x, init_state, adj_matrix, Wg, bg, Wu, bu)
    res = run_bass_kernel_spmd(nc, in_maps, core_ids=list(range(N_CORES)),
                               trace=True, tmpdir=tmpdir)
    return decode_outputs(res.results), res
